# revision 22
# baseline (speedup 1.0000x reference)
"""Causal self-attention (B=2, T=2048, C=1024, H=16) on 8 TRN2 NeuronCores.

Sharding: data-parallel over batch (2 groups of 4 cores) x tensor-parallel
over heads (4 heads per core). Each core computes qkv for its 4 heads,
causal attention, and a partial output projection (y_heads @ w_proj rows).
Host sums the 4 partial projections per batch and adds b_proj.

Key implementation points (vs the straightforward version):
  - q/k projections run in fp8 e4m3 DoubleRow mode (2 k-subtiles per pass);
    weights are pre-scaled by 64 host-side to stay in e4m3 normal range and
    the 64*64 factor is folded into the exp() scale. v stays bf16.
  - The causal mask is applied on the PE (a -1e9*I @ tri matmul accumulated
    into the score PSUM) instead of a vector op, keeping the PE->ACT->PE
    chain free of DVE hops.
  - v bias is folded into the v PSUM->SBUF copy; attention output bias is
    exact because softmax weights sum to 1.
  - Emission is software-pipelined: qkv for chunk j+1 and projection tiles
    for chunk j-1 are interleaved piecewise into the attention steps of
    chunk j so the PE never starves while the scalar engine runs exp.
  - Partial projections DMA out in bf16 (summed in f32 host-side).
"""
import threading

import numpy as np

import concourse.bass as bass
import concourse.tile as tile
from concourse import bacc, mybir
from concourse.bass2jax import (
    _bass_exec_p,
    install_neuronx_cc_hook,
    partition_id_tensor,
)

N_CORES = 8
B, T, C, H = 2, 2048, 1024, 16
D = C // H            # 64
HL = 4                # heads per core
G = H // HL           # 4 head groups
SC = 1.0 / (32.0 * 4096.0)   # 1/sqrt(C) / (64*64 weight prescale)
F32 = mybir.dt.float32
BF16 = mybir.dt.bfloat16
FP8 = mybir.dt.float8e4
NEG = -1e9
WSCALE = 64.0


def build_nc():
    nc = bacc.Bacc("TRN2", target_bir_lowering=False, debug=False,
                   num_devices=N_CORES)
    x = nc.dram_tensor("x", [128, 4, 8, 512], BF16, kind="ExternalInput").ap()
    x8 = nc.dram_tensor("x8", [128, 4, 8, 512], FP8,
                        kind="ExternalInput").ap()
    wq = nc.dram_tensor("wq", [128, 8, 256], FP8, kind="ExternalInput").ap()
    wk = nc.dram_tensor("wk", [128, 8, 256], FP8, kind="ExternalInput").ap()
    wv = nc.dram_tensor("wv", [128, 8, 256], BF16, kind="ExternalInput").ap()
    bq = nc.dram_tensor("bq", [128, 2], F32, kind="ExternalInput").ap()
    bk = nc.dram_tensor("bk", [128, 2], F32, kind="ExternalInput").ap()
    bv = nc.dram_tensor("bv", [1, 256], F32, kind="ExternalInput").ap()
    wp = nc.dram_tensor("wp", [128, 2, C], BF16, kind="ExternalInput").ap()
    tri = nc.dram_tensor("tri", [128, 2, 128], BF16, kind="ExternalInput").ap()
    neye = nc.dram_tensor("neye", [128, 128], BF16, kind="ExternalInput").ap()
    out = nc.dram_tensor("out", [T, C], BF16, kind="ExternalOutput").ap()

    Exp = mybir.ActivationFunctionType.Exp
    add = mybir.AluOpType.add
    mult = mybir.AluOpType.mult
    DR = mybir.MatmulPerfMode.DoubleRow

    with tile.TileContext(nc) as tc:
        with tc.tile_pool(name="const", bufs=1) as cp, \
             tc.tile_pool(name="ps", bufs=2, space="PSUM") as psp, \
             tc.tile_pool(name="sT", bufs=2, space="PSUM") as stp, \
             tc.tile_pool(name="pvps", bufs=2, space="PSUM") as pvp, \
             tc.tile_pool(name="ptp", bufs=6) as ptp, \
             tc.tile_pool(name="recp", bufs=6) as recp, \
             tc.tile_pool(name="outp", bufs=3) as outp:

            # ---- persistent sbuf (x stored chunk-major: [p, chunk, kt, q])
            xt = cp.tile([128, 4, 8, 512], BF16, tag="xt")   # bf16 (for v)
            x8t = cp.tile([128, 4, 8, 512], FP8, tag="x8t")  # fp8 (for q/k)
            qT = cp.tile([128, 2, T], BF16, tag="qT")        # pair-stacked
            kT = cp.tile([128, 2, T], BF16, tag="kT")
            va = cp.tile([128, 16, HL, D + 1], BF16, tag="va")  # v + ones col
            yt = cp.tile([128, 2, T], BF16, tag="yt")
            wq8 = cp.tile([128, 8, 256], FP8, tag="wq8")
            wk8 = cp.tile([128, 8, 256], FP8, tag="wk8")
            wv16 = cp.tile([128, 8, 256], BF16, tag="wv16")
            wp16 = cp.tile([128, 2, C], BF16, tag="wp16")
            bq_sb = cp.tile([128, 2], F32, tag="bq_sb")
            bk_sb = cp.tile([128, 2], F32, tag="bk_sb")
            bv_row = cp.tile([1, 256], F32, tag="bv_row")
            bv_bc = cp.tile([128, HL, D], F32, tag="bv_bc")
            tri_sb = cp.tile([128, 2, 128], BF16, tag="tri_sb")
            neye_sb = cp.tile([128, 128], BF16, tag="neye_sb")

            # ---- phase 0: loads (inputs pre-cast/pre-swizzled host-side) ----
            # gpsimd queue: weights/consts; sync queue: x streams
            # critical path first: x8 chunk 0 split across two queues, q/k
            # weights on a third; bulk xt/x8 streams follow behind.
            nc.sync.dma_start(x8t[:, 0, 0:4], x8[:, 0, 0:4])
            nc.scalar.dma_start(x8t[:, 0, 4:8], x8[:, 0, 4:8])
            nc.gpsimd.dma_start(wq8[:], wq[:])
            nc.gpsimd.dma_start(wk8[:], wk[:])
            nc.gpsimd.dma_start(wv16[:], wv[:])
            nc.scalar.dma_start(xt[:, 0], x[:, 0])
            for c in range(1, 4):
                nc.sync.dma_start(x8t[:, c], x8[:, c])
            for c in range(1, 4):
                nc.sync.dma_start(xt[:, c], x[:, c])
            nc.gpsimd.dma_start(tri_sb[:], tri[:])
            nc.gpsimd.dma_start(neye_sb[:], neye[:])
            nc.gpsimd.dma_start(bq_sb[:], bq[:])
            nc.gpsimd.dma_start(bk_sb[:], bk[:])
            nc.gpsimd.dma_start(bv_row[:], bv[:])
            nc.gpsimd.partition_broadcast(
                bv_bc[:].rearrange("p h d -> p (h d)"), bv_row[:])
            nc.gpsimd.dma_start(wp16[:], wp[:])
            nc.vector.memset(va[:, :, :, D:D + 1], 1.0)

            def qk_piece(w8, bias_sb, dst, p, j):
                qs = slice(512 * j, 512 * (j + 1))
                psq = psp.tile([128, 512], F32, tag="ps")
                for dr in range(4):
                    nc.tensor.matmul(
                        psq[:],
                        w8[:, 2 * dr:2 * dr + 2, 128 * p:128 * (p + 1)],
                        x8t[:, j, 2 * dr:2 * dr + 2, :],
                        start=(dr == 0), stop=(dr == 3), perf_mode=DR)
                nc.vector.tensor_scalar_add(
                    dst[:, p, qs], psq[:], bias_sb[:, p:p + 1])

            def v_piece(t):
                psv = psp.tile([128, 512], F32, tag="ps")
                for kt_i in range(8):
                    nc.tensor.matmul(
                        psv[:, :256],
                        xt[:, t // 4, kt_i, 128 * (t % 4):128 * (t % 4 + 1)],
                        wv16[:, kt_i, :],
                        start=(kt_i == 0), stop=(kt_i == 7))
                nc.vector.tensor_tensor(
                    va[:, t, :, 0:D],
                    psv[:, :256].rearrange("p (h d) -> p h d", h=HL),
                    bv_bc[:], add)

            def qkv_pieces(j):
                ps = []
                for (w8, bias_sb, dst) in ((wq8, bq_sb, qT), (wk8, bk_sb, kT)):
                    for p in range(2):
                        ps.append(lambda w8=w8, b=bias_sb, d=dst, p=p:
                                  qk_piece(w8, b, d, p, j))
                for t in range(4 * j, 4 * (j + 1)):
                    ps.append(lambda t=t: v_piece(t))
                return ps

            Copy = mybir.ActivationFunctionType.Copy

            def proj_pieces(j, tail=False):
                def proj_tile(t):
                    osb = outp.tile([128, C], BF16, tag="osb")
                    for cc in range(2):
                        ops = psp.tile([128, 512], F32, tag="ps")
                        for u in range(2):
                            nc.tensor.matmul(
                                ops[:],
                                yt[:, u, 128 * t:128 * (t + 1)],
                                wp16[:, u, 512 * cc:512 * (cc + 1)],
                                start=(u == 0), stop=(u == 1))
                        dst = osb[:, 512 * cc:512 * (cc + 1)]
                        if tail and cc == 1:
                            nc.scalar.activation(dst, ops[:], Copy)
                        else:
                            nc.vector.tensor_copy(out=dst, in_=ops[:])
                    nc.sync.dma_start(out[128 * t:128 * (t + 1), :], osb[:])
                return [lambda t=t: proj_tile(t)
                        for t in range(4 * j, 4 * (j + 1))]

            # ---- chunk 0 q/k up front (v tiles ride the stuffer) ----
            qk_piece(wq8, bq_sb, qT, 0, 0)
            qk_piece(wk8, bk_sb, kT, 0, 0)
            qk_piece(wq8, bq_sb, qT, 1, 0)
            qk_piece(wk8, bk_sb, kT, 1, 0)

            # ---- attention chunks with interleaved qkv/proj stuffing ----
            for j in range(4):
                stuff = []
                if j == 0:
                    stuff += [lambda t=t: v_piece(t) for t in range(4)]
                if j < 3:
                    stuff += qkv_pieces(j + 1)
                if j >= 1:
                    stuff += proj_pieces(j - 1)
                nkb = 4 * j + 4
                nsteps = 2 * nkb
                step = 0
                emitted = 0
                for p in range(2):
                    pvs = [pvp.tile([65, 512], F32, tag="pv", name=f"pv{_h}")
                           for _h in range(2)]
                    pending = []
                    for kb in range(nkb):
                        off = 128 * (kb - 4 * j) if kb >= 4 * j else 0
                        s2 = stp.tile([128, 2, 512], F32, tag="sT")
                        for hh in range(2):
                            pr = 64 * hh
                            nc.tensor.matmul(
                                s2[:, hh, off:512],
                                kT[pr:pr + 64, p, 128 * kb:128 * (kb + 1)],
                                qT[pr:pr + 64, p,
                                   512 * j + off:512 * (j + 1)],
                                start=True, stop=True)
                        if kb >= 4 * j:
                            nc.tensor.matmul(
                                s2[:, :, off:off + 128],
                                neye_sb[:], tri_sb[:],
                                start=False, stop=True,
                                skip_group_check=True)
                        ptt = ptp.tile([128, 2, 512], BF16, tag="pt")
                        nc.scalar.activation(ptt[:, :, off:512],
                                             s2[:, :, off:512], Exp, scale=SC)
                        pending.append((kb, off, ptt))
                        step += 1
                        while emitted * nsteps < len(stuff) * step:
                            stuff[emitted]()
                            emitted += 1
                        if len(pending) > 1:
                            pkb, poff, pptt = pending.pop(0)
                            for hh in range(2):
                                nc.tensor.matmul(
                                    pvs[hh][:, poff:512],
                                    va[:, pkb, 2 * p + hh, :],
                                    pptt[:, hh, poff:512],
                                    start=(pkb == 0), stop=False)
                    for i, (pkb, poff, pptt) in enumerate(pending):
                        for hh in range(2):
                            nc.tensor.matmul(
                                pvs[hh][:, poff:512],
                                va[:, pkb, 2 * p + hh, :],
                                pptt[:, hh, poff:512],
                                start=(pkb == 0),
                                stop=(i == len(pending) - 1))
                    # normalize: yT = y' * (1/rowsum) broadcast over d
                    # (recips first, then broadcasts, then mults, so the
                    # DVE->gpsimd->DVE chains of both heads overlap)
                    recs, bcs = [], []
                    for hh in range(2):
                        rowf = recp.tile([1, 512], F32, tag=f"rowf{hh}")
                        nc.vector.tensor_copy(out=rowf[:],
                                              in_=pvs[hh][64:65, :])
                        rec1 = recp.tile([1, 512], F32, tag=f"rec1{hh}")
                        nc.vector.reciprocal_approx_fast(rec1[:], rowf[:])
                        recs.append(rec1)
                    for hh in range(2):
                        bc = recp.tile([64, 512], F32, tag=f"bc{hh}")
                        nc.gpsimd.partition_broadcast(bc[:], recs[hh][:])
                        bcs.append(bc)
                    for hh in range(2):
                        nc.vector.tensor_tensor(
                            yt[64 * hh:64 * hh + 64, p,
                               512 * j:512 * (j + 1)],
                            pvs[hh][0:64, :], bcs[hh][:], mult)
                while emitted < len(stuff):
                    stuff[emitted]()
                    emitted += 1
            for piece in proj_pieces(3, tail=True):
                piece()

    nc.compile()
    return nc


def make_fn(nc):
    """Sharded 8-core jit callable for the compiled Bass program."""
    import jax
    from jax.sharding import Mesh, PartitionSpec
    from jax.experimental.shard_map import shard_map

    install_neuronx_cc_hook()
    in_names, out_names, out_avals, zero_outs = [], [], [], []
    pname = nc.partition_id_tensor.name if nc.partition_id_tensor else None
    for alloc in nc.m.functions[0].allocations:
        if not isinstance(alloc, mybir.MemoryLocationSet):
            continue
        name = alloc.memorylocations[0].name
        if alloc.kind == "ExternalInput":
            if name != pname:
                in_names.append(name)
        elif alloc.kind == "ExternalOutput":
            out_names.append(name)
            shape = tuple(alloc.tensor_shape)
            dtype = mybir.dt.np(alloc.dtype)
            out_avals.append(jax.core.ShapedArray(shape, dtype))
            zero_outs.append(np.zeros(shape, dtype))
    n_params = len(in_names)
    all_names = list(in_names) + out_names
    if pname is not None:
        all_names.append(pname)

    def _body(*args):
        operands = list(args)
        if pname is not None:
            operands.append(partition_id_tensor())
        outs = _bass_exec_p.bind(
            *operands, out_avals=tuple(out_avals), in_names=tuple(all_names),
            out_names=tuple(out_names), lowering_input_output_aliases=(),
            sim_require_finite=True, sim_require_nnan=True, nc=nc)
        return tuple(outs)

    devices = jax.devices()[:N_CORES]
    mesh = Mesh(np.asarray(devices), ("core",))
    n_out = len(out_names)
    fn = jax.jit(
        shard_map(_body, mesh=mesh,
                  in_specs=(PartitionSpec("core"),) * (n_params + n_out),
                  out_specs=(PartitionSpec("core"),) * n_out,
                  check_rep=False),
        keep_unused=True)
    return fn, in_names, out_names, zero_outs


def shard_inputs(x, w_attn, b_attn, w_proj, b_proj):
    """Build the per-core input maps (core = 4*batch + head_group).

    Host-side prep is layout only: slicing per core, bf16/fp8 rounding, and
    the [128, ko, n] partition swizzle the device matmuls consume."""
    import ml_dtypes
    bf16 = ml_dtypes.bfloat16
    f8 = ml_dtypes.float8_e4m3
    x = np.asarray(x, dtype=np.float32)
    w_attn = np.asarray(w_attn, dtype=np.float32)
    b_attn = np.asarray(b_attn, dtype=np.float32)
    w_proj = np.asarray(w_proj, dtype=np.float32)
    tri = np.where(np.arange(128)[:, None] > np.arange(128)[None, :],
                   np.float32(1.0), np.float32(0.0)).astype(bf16)
    tri2 = np.ascontiguousarray(
        np.broadcast_to(tri[:, None, :], (128, 2, 128)))
    neye = (np.eye(128, dtype=np.float32) * np.float32(NEG)).astype(bf16)
    swz = lambda w, dt: np.ascontiguousarray(
        w.reshape(-1, 128, w.shape[1]).transpose(1, 0, 2).astype(dt))
    # chunk-major x: [p, chunk, kt, q] so per-chunk DMA is contiguous/part
    xsw = [np.ascontiguousarray(
        x[b].T.reshape(8, 128, 4, 512).transpose(1, 2, 0, 3))
        for b in range(B)]
    x16 = [v.astype(bf16) for v in xsw]
    x8 = [v.astype(f8) for v in xsw]
    in_maps = []
    for core in range(N_CORES):
        b, g = divmod(core, G)
        cs = slice(256 * g, 256 * (g + 1))
        bcol = lambda v: np.ascontiguousarray(v.reshape(2, 128).T)
        in_maps.append({
            "x": x16[b],
            "x8": x8[b],
            "wq": swz(w_attn[:, cs] * WSCALE, f8),
            "wk": swz(w_attn[:, 1024:][:, cs] * WSCALE, f8),
            "wv": swz(w_attn[:, 2048:][:, cs], bf16),
            "bq": bcol(b_attn[cs] * WSCALE),
            "bk": bcol(b_attn[1024:][cs] * WSCALE),
            "bv": np.ascontiguousarray(b_attn[2048:][cs]).reshape(1, 256),
            "wp": swz(w_proj[cs, :], bf16),
            "tri": tri2,
            "neye": neye,
        })
    return in_maps


_cache = {}
_lock = threading.Lock()


def _get_compiled():
    with _lock:
        if "fn" not in _cache:
            nc = build_nc()
            fn, in_names, out_names, zero_outs = make_fn(nc)
            _cache.update(fn=fn, nc=nc, in_names=in_names,
                          out_names=out_names, zero_outs=zero_outs)
    return _cache


def run_cores(in_maps):
    """Execute the 8-core SPMD program, return per-core output dicts."""
    import jax

    cc = _get_compiled()
    concat_in = [np.concatenate([m[k] for m in in_maps], axis=0)
                 for k in cc["in_names"]]
    concat_zeros = [np.zeros((N_CORES * z.shape[0], *z.shape[1:]), z.dtype)
                    for z in cc["zero_outs"]]
    outs = cc["fn"](*[jax.device_put(v) for v in concat_in],
                    *[jax.device_put(z) for z in concat_zeros])
    res = []
    for c in range(N_CORES):
        res.append({name: np.asarray(outs[i]).reshape(
            N_CORES, *cc["zero_outs"][i].shape)[c]
            for i, name in enumerate(cc["out_names"])})
    return res


def kernel(x, w_attn, b_attn, w_proj, b_proj):
    in_maps = shard_inputs(x, w_attn, b_attn, w_proj, b_proj)
    res = run_cores(in_maps)
    b_proj = np.asarray(b_proj, dtype=np.float32)
    out = np.empty((B, T, C), dtype=np.float32)
    for b in range(B):
        acc = res[4 * b]["out"].astype(np.float32)
        for g in range(1, G):
            acc = acc + res[4 * b + g]["out"].astype(np.float32)
        out[b] = acc + b_proj
    return out


# revision 25
# speedup vs baseline: 1.0274x; 1.0274x over previous
"""Causal self-attention (B=2, T=2048, C=1024, H=16) on 8 TRN2 NeuronCores.

Sharding: data-parallel over batch (2 groups of 4 cores) x tensor-parallel
over heads (4 heads per core). Each core computes qkv for its 4 heads,
causal attention, and a partial output projection (y_heads @ w_proj rows).
Host sums the 4 partial projections per batch and adds b_proj.

Key implementation points (vs the straightforward version):
  - q/k projections run in fp8 e4m3 DoubleRow mode (2 k-subtiles per pass);
    weights are pre-scaled by 64 host-side to stay in e4m3 normal range and
    the 64*64 factor is folded into the exp() scale. v stays bf16.
  - The causal mask is applied on the PE (a -1e9*I @ tri matmul accumulated
    into the score PSUM) instead of a vector op, keeping the PE->ACT->PE
    chain free of DVE hops.
  - v bias is folded into the v PSUM->SBUF copy; attention output bias is
    exact because softmax weights sum to 1.
  - Emission is software-pipelined: qkv for chunk j+1 and projection tiles
    for chunk j-1 are interleaved piecewise into the attention steps of
    chunk j so the PE never starves while the scalar engine runs exp.
  - Partial projections DMA out in bf16 (summed in f32 host-side).
"""
import threading

import numpy as np

import concourse.bass as bass
import concourse.tile as tile
from concourse import bacc, mybir
from concourse.bass2jax import (
    _bass_exec_p,
    install_neuronx_cc_hook,
    partition_id_tensor,
)

N_CORES = 8
B, T, C, H = 2, 2048, 1024, 16
D = C // H            # 64
HL = 4                # heads per core
G = H // HL           # 4 head groups
SC = 1.0 / (32.0 * 4096.0)   # 1/sqrt(C) / (64*64 weight prescale)
F32 = mybir.dt.float32
BF16 = mybir.dt.bfloat16
FP8 = mybir.dt.float8e4
NEG = -1e9
WSCALE = 64.0


def build_nc():
    nc = bacc.Bacc("TRN2", target_bir_lowering=False, debug=False,
                   num_devices=N_CORES)
    x = nc.dram_tensor("x", [128, 4, 8, 512], BF16, kind="ExternalInput").ap()
    x8 = nc.dram_tensor("x8", [128, 4, 8, 512], FP8,
                        kind="ExternalInput").ap()
    wq = nc.dram_tensor("wq", [128, 8, 256], FP8, kind="ExternalInput").ap()
    wk = nc.dram_tensor("wk", [128, 8, 256], FP8, kind="ExternalInput").ap()
    wv = nc.dram_tensor("wv", [128, 8, 256], BF16, kind="ExternalInput").ap()
    bq = nc.dram_tensor("bq", [128, 2], F32, kind="ExternalInput").ap()
    bk = nc.dram_tensor("bk", [128, 2], F32, kind="ExternalInput").ap()
    bv = nc.dram_tensor("bv", [1, 256], F32, kind="ExternalInput").ap()
    wp = nc.dram_tensor("wp", [128, 2, C], BF16, kind="ExternalInput").ap()
    tri = nc.dram_tensor("tri", [128, 2, 128], BF16, kind="ExternalInput").ap()
    neye = nc.dram_tensor("neye", [128, 128], BF16, kind="ExternalInput").ap()
    out = nc.dram_tensor("out", [T, C], BF16, kind="ExternalOutput").ap()

    Exp = mybir.ActivationFunctionType.Exp
    add = mybir.AluOpType.add
    mult = mybir.AluOpType.mult
    DR = mybir.MatmulPerfMode.DoubleRow

    with tile.TileContext(nc) as tc:
        with tc.tile_pool(name="const", bufs=1) as cp, \
             tc.tile_pool(name="ps", bufs=2, space="PSUM") as psp, \
             tc.tile_pool(name="sT", bufs=2, space="PSUM") as stp, \
             tc.tile_pool(name="pvps", bufs=2, space="PSUM") as pvp, \
             tc.tile_pool(name="ptp", bufs=6) as ptp, \
             tc.tile_pool(name="recp", bufs=6) as recp, \
             tc.tile_pool(name="outp", bufs=3) as outp:

            # ---- persistent sbuf (x stored chunk-major: [p, chunk, kt, q])
            xt = cp.tile([128, 4, 8, 512], BF16, tag="xt")   # bf16 (for v)
            x8t = cp.tile([128, 4, 8, 512], FP8, tag="x8t")  # fp8 (for q/k)
            qT = cp.tile([128, 2, T], BF16, tag="qT")        # pair-stacked
            kT = cp.tile([128, 2, T], BF16, tag="kT")
            va = cp.tile([128, 16, HL, D + 1], BF16, tag="va")  # v + ones col
            yt = cp.tile([128, 2, T], BF16, tag="yt")
            wq8 = cp.tile([128, 8, 256], FP8, tag="wq8")
            wk8 = cp.tile([128, 8, 256], FP8, tag="wk8")
            wv16 = cp.tile([128, 8, 256], BF16, tag="wv16")
            wp16 = cp.tile([128, 2, C], BF16, tag="wp16")
            bq_sb = cp.tile([128, 2], F32, tag="bq_sb")
            bk_sb = cp.tile([128, 2], F32, tag="bk_sb")
            bv_row = cp.tile([1, 256], F32, tag="bv_row")
            bv_bc = cp.tile([128, HL, D], F32, tag="bv_bc")
            tri_sb = cp.tile([128, 2, 128], BF16, tag="tri_sb")
            neye_sb = cp.tile([128, 128], BF16, tag="neye_sb")

            # ---- phase 0: loads (inputs pre-cast/pre-swizzled host-side) ----
            # gpsimd queue: weights/consts; sync queue: x streams
            # critical path first: x8 chunk 0 in quarters across three
            # queues, q/k weights + tiny consts on a fourth; bulk xt/x8
            # streams follow behind.
            nc.sync.dma_start(x8t[:, 0, 0:4], x8[:, 0, 0:4])
            nc.scalar.dma_start(x8t[:, 0, 4:8], x8[:, 0, 4:8])
            nc.gpsimd.dma_start(wq8[:], wq[:])
            nc.gpsimd.dma_start(wk8[:], wk[:])
            nc.gpsimd.dma_start(tri_sb[:], tri[:])
            nc.gpsimd.dma_start(neye_sb[:], neye[:])
            nc.gpsimd.dma_start(bq_sb[:], bq[:])
            nc.gpsimd.dma_start(bk_sb[:], bk[:])
            nc.gpsimd.dma_start(bv_row[:], bv[:])
            nc.gpsimd.dma_start(wv16[:], wv[:])
            nc.scalar.dma_start(xt[:, 0], x[:, 0])
            for c in range(1, 4):
                nc.sync.dma_start(x8t[:, c], x8[:, c])
            for c in range(1, 4):
                nc.sync.dma_start(xt[:, c], x[:, c])
            nc.gpsimd.partition_broadcast(
                bv_bc[:].rearrange("p h d -> p (h d)"), bv_row[:])
            nc.gpsimd.dma_start(wp16[:], wp[:])
            nc.vector.memset(va[:, :, :, D:D + 1], 1.0)

            def qk_piece(w8, bias_sb, dst, p, j):
                qs = slice(512 * j, 512 * (j + 1))
                psq = psp.tile([128, 512], F32, tag="ps")
                for dr in range(4):
                    nc.tensor.matmul(
                        psq[:],
                        w8[:, 2 * dr:2 * dr + 2, 128 * p:128 * (p + 1)],
                        x8t[:, j, 2 * dr:2 * dr + 2, :],
                        start=(dr == 0), stop=(dr == 3), perf_mode=DR)
                nc.vector.tensor_scalar_add(
                    dst[:, p, qs], psq[:], bias_sb[:, p:p + 1])

            def v_piece(t):
                psv = psp.tile([128, 512], F32, tag="ps")
                for kt_i in range(8):
                    nc.tensor.matmul(
                        psv[:, :256],
                        xt[:, t // 4, kt_i, 128 * (t % 4):128 * (t % 4 + 1)],
                        wv16[:, kt_i, :],
                        start=(kt_i == 0), stop=(kt_i == 7))
                nc.vector.tensor_tensor(
                    va[:, t, :, 0:D],
                    psv[:, :256].rearrange("p (h d) -> p h d", h=HL),
                    bv_bc[:], add)

            def qkv_pieces(j):
                ps = []
                for (w8, bias_sb, dst) in ((wq8, bq_sb, qT), (wk8, bk_sb, kT)):
                    for p in range(2):
                        ps.append(lambda w8=w8, b=bias_sb, d=dst, p=p:
                                  qk_piece(w8, b, d, p, j))
                for t in range(4 * j, 4 * (j + 1)):
                    ps.append(lambda t=t: v_piece(t))
                return ps

            Copy = mybir.ActivationFunctionType.Copy

            def proj_pieces(j, tail=False):
                def proj_tile(t):
                    osb = outp.tile([128, C], BF16, tag="osb")
                    for cc in range(2):
                        ops = psp.tile([128, 512], F32, tag="ps")
                        for u in range(2):
                            nc.tensor.matmul(
                                ops[:],
                                yt[:, u, 128 * t:128 * (t + 1)],
                                wp16[:, u, 512 * cc:512 * (cc + 1)],
                                start=(u == 0), stop=(u == 1))
                        dst = osb[:, 512 * cc:512 * (cc + 1)]
                        osl = out[128 * t:128 * (t + 1),
                                  512 * cc:512 * (cc + 1)]
                        if tail and cc == 1:
                            nc.scalar.activation(dst, ops[:], Copy)
                            nc.scalar.dma_start(osl, dst)
                        elif tail:
                            nc.vector.tensor_copy(out=dst, in_=ops[:])
                            nc.sync.dma_start(osl, dst)
                        else:
                            nc.vector.tensor_copy(out=dst, in_=ops[:])
                    if not tail:
                        nc.sync.dma_start(out[128 * t:128 * (t + 1), :],
                                          osb[:])
                return [lambda t=t: proj_tile(t)
                        for t in range(4 * j, 4 * (j + 1))]

            # ---- chunk 0 q/k up front (v tiles ride the stuffer) ----
            qk_piece(wq8, bq_sb, qT, 0, 0)
            qk_piece(wk8, bk_sb, kT, 0, 0)
            qk_piece(wq8, bq_sb, qT, 1, 0)
            qk_piece(wk8, bk_sb, kT, 1, 0)

            # ---- attention chunks with interleaved qkv/proj stuffing ----
            for j in range(4):
                stuff = []
                if j == 0:
                    stuff += [lambda t=t: v_piece(t) for t in range(4)]
                if j < 3:
                    stuff += qkv_pieces(j + 1)
                if j >= 1:
                    stuff += proj_pieces(j - 1)
                nkb = 4 * j + 4
                nsteps = 2 * nkb
                step = 0
                emitted = 0
                for p in range(2):
                    pvs = [pvp.tile([65, 512], F32, tag="pv", name=f"pv{_h}")
                           for _h in range(2)]
                    pending = []
                    for kb in range(nkb):
                        off = 128 * (kb - 4 * j) if kb >= 4 * j else 0
                        s2 = stp.tile([128, 2, 512], F32, tag="sT")
                        for hh in range(2):
                            pr = 64 * hh
                            nc.tensor.matmul(
                                s2[:, hh, off:512],
                                kT[pr:pr + 64, p, 128 * kb:128 * (kb + 1)],
                                qT[pr:pr + 64, p,
                                   512 * j + off:512 * (j + 1)],
                                start=True, stop=True)
                        if kb >= 4 * j:
                            nc.tensor.matmul(
                                s2[:, :, off:off + 128],
                                neye_sb[:], tri_sb[:],
                                start=False, stop=True,
                                skip_group_check=True)
                        ptt = ptp.tile([128, 2, 512], BF16, tag="pt")
                        nc.scalar.activation(ptt[:, :, off:512],
                                             s2[:, :, off:512], Exp, scale=SC)
                        pending.append((kb, off, ptt))
                        step += 1
                        while emitted * nsteps < len(stuff) * step:
                            stuff[emitted]()
                            emitted += 1
                        if len(pending) > 1:
                            pkb, poff, pptt = pending.pop(0)
                            for hh in range(2):
                                nc.tensor.matmul(
                                    pvs[hh][:, poff:512],
                                    va[:, pkb, 2 * p + hh, :],
                                    pptt[:, hh, poff:512],
                                    start=(pkb == 0), stop=False)
                    for i, (pkb, poff, pptt) in enumerate(pending):
                        for hh in range(2):
                            nc.tensor.matmul(
                                pvs[hh][:, poff:512],
                                va[:, pkb, 2 * p + hh, :],
                                pptt[:, hh, poff:512],
                                start=(pkb == 0),
                                stop=(i == len(pending) - 1))
                    # normalize: yT = y' * (1/rowsum) broadcast over d
                    # (recips first, then broadcasts, then mults, so the
                    # DVE->gpsimd->DVE chains of both heads overlap)
                    recs, bcs = [], []
                    for hh in range(2):
                        rowf = recp.tile([1, 512], F32, tag=f"rowf{hh}")
                        nc.vector.tensor_copy(out=rowf[:],
                                              in_=pvs[hh][64:65, :])
                        rec1 = recp.tile([1, 512], F32, tag=f"rec1{hh}")
                        nc.vector.reciprocal_approx_fast(rec1[:], rowf[:])
                        recs.append(rec1)
                    for hh in range(2):
                        bc = recp.tile([64, 512], F32, tag=f"bc{hh}")
                        nc.gpsimd.partition_broadcast(bc[:], recs[hh][:])
                        bcs.append(bc)
                    for hh in range(2):
                        nc.vector.tensor_tensor(
                            yt[64 * hh:64 * hh + 64, p,
                               512 * j:512 * (j + 1)],
                            pvs[hh][0:64, :], bcs[hh][:], mult)
                while emitted < len(stuff):
                    stuff[emitted]()
                    emitted += 1
            for piece in proj_pieces(3, tail=True):
                piece()

    nc.compile()
    return nc


def make_fn(nc):
    """Sharded 8-core jit callable for the compiled Bass program."""
    import jax
    from jax.sharding import Mesh, PartitionSpec
    from jax.experimental.shard_map import shard_map

    install_neuronx_cc_hook()
    in_names, out_names, out_avals, zero_outs = [], [], [], []
    pname = nc.partition_id_tensor.name if nc.partition_id_tensor else None
    for alloc in nc.m.functions[0].allocations:
        if not isinstance(alloc, mybir.MemoryLocationSet):
            continue
        name = alloc.memorylocations[0].name
        if alloc.kind == "ExternalInput":
            if name != pname:
                in_names.append(name)
        elif alloc.kind == "ExternalOutput":
            out_names.append(name)
            shape = tuple(alloc.tensor_shape)
            dtype = mybir.dt.np(alloc.dtype)
            out_avals.append(jax.core.ShapedArray(shape, dtype))
            zero_outs.append(np.zeros(shape, dtype))
    n_params = len(in_names)
    all_names = list(in_names) + out_names
    if pname is not None:
        all_names.append(pname)

    def _body(*args):
        operands = list(args)
        if pname is not None:
            operands.append(partition_id_tensor())
        outs = _bass_exec_p.bind(
            *operands, out_avals=tuple(out_avals), in_names=tuple(all_names),
            out_names=tuple(out_names), lowering_input_output_aliases=(),
            sim_require_finite=True, sim_require_nnan=True, nc=nc)
        return tuple(outs)

    devices = jax.devices()[:N_CORES]
    mesh = Mesh(np.asarray(devices), ("core",))
    n_out = len(out_names)
    fn = jax.jit(
        shard_map(_body, mesh=mesh,
                  in_specs=(PartitionSpec("core"),) * (n_params + n_out),
                  out_specs=(PartitionSpec("core"),) * n_out,
                  check_rep=False),
        keep_unused=True)
    return fn, in_names, out_names, zero_outs


def shard_inputs(x, w_attn, b_attn, w_proj, b_proj):
    """Build the per-core input maps (core = 4*batch + head_group).

    Host-side prep is layout only: slicing per core, bf16/fp8 rounding, and
    the [128, ko, n] partition swizzle the device matmuls consume."""
    import ml_dtypes
    bf16 = ml_dtypes.bfloat16
    f8 = ml_dtypes.float8_e4m3
    x = np.asarray(x, dtype=np.float32)
    w_attn = np.asarray(w_attn, dtype=np.float32)
    b_attn = np.asarray(b_attn, dtype=np.float32)
    w_proj = np.asarray(w_proj, dtype=np.float32)
    tri = np.where(np.arange(128)[:, None] > np.arange(128)[None, :],
                   np.float32(1.0), np.float32(0.0)).astype(bf16)
    tri2 = np.ascontiguousarray(
        np.broadcast_to(tri[:, None, :], (128, 2, 128)))
    neye = (np.eye(128, dtype=np.float32) * np.float32(NEG)).astype(bf16)
    swz = lambda w, dt: np.ascontiguousarray(
        w.reshape(-1, 128, w.shape[1]).transpose(1, 0, 2).astype(dt))
    # chunk-major x: [p, chunk, kt, q] so per-chunk DMA is contiguous/part
    xsw = [np.ascontiguousarray(
        x[b].T.reshape(8, 128, 4, 512).transpose(1, 2, 0, 3))
        for b in range(B)]
    x16 = [v.astype(bf16) for v in xsw]
    x8 = [v.astype(f8) for v in xsw]
    in_maps = []
    for core in range(N_CORES):
        b, g = divmod(core, G)
        cs = slice(256 * g, 256 * (g + 1))
        bcol = lambda v: np.ascontiguousarray(v.reshape(2, 128).T)
        in_maps.append({
            "x": x16[b],
            "x8": x8[b],
            "wq": swz(w_attn[:, cs] * WSCALE, f8),
            "wk": swz(w_attn[:, 1024:][:, cs] * WSCALE, f8),
            "wv": swz(w_attn[:, 2048:][:, cs], bf16),
            "bq": bcol(b_attn[cs] * WSCALE),
            "bk": bcol(b_attn[1024:][cs] * WSCALE),
            "bv": np.ascontiguousarray(b_attn[2048:][cs]).reshape(1, 256),
            "wp": swz(w_proj[cs, :], bf16),
            "tri": tri2,
            "neye": neye,
        })
    return in_maps


_cache = {}
_lock = threading.Lock()


def _get_compiled():
    with _lock:
        if "fn" not in _cache:
            nc = build_nc()
            fn, in_names, out_names, zero_outs = make_fn(nc)
            _cache.update(fn=fn, nc=nc, in_names=in_names,
                          out_names=out_names, zero_outs=zero_outs)
    return _cache


def run_cores(in_maps):
    """Execute the 8-core SPMD program, return per-core output dicts."""
    import jax

    cc = _get_compiled()
    concat_in = [np.concatenate([m[k] for m in in_maps], axis=0)
                 for k in cc["in_names"]]
    concat_zeros = [np.zeros((N_CORES * z.shape[0], *z.shape[1:]), z.dtype)
                    for z in cc["zero_outs"]]
    outs = cc["fn"](*[jax.device_put(v) for v in concat_in],
                    *[jax.device_put(z) for z in concat_zeros])
    res = []
    for c in range(N_CORES):
        res.append({name: np.asarray(outs[i]).reshape(
            N_CORES, *cc["zero_outs"][i].shape)[c]
            for i, name in enumerate(cc["out_names"])})
    return res


def kernel(x, w_attn, b_attn, w_proj, b_proj):
    in_maps = shard_inputs(x, w_attn, b_attn, w_proj, b_proj)
    res = run_cores(in_maps)
    b_proj = np.asarray(b_proj, dtype=np.float32)
    out = np.empty((B, T, C), dtype=np.float32)
    for b in range(B):
        acc = res[4 * b]["out"].astype(np.float32)
        for g in range(1, G):
            acc = acc + res[4 * b + g]["out"].astype(np.float32)
        out[b] = acc + b_proj
    return out


# revision 26
# speedup vs baseline: 1.0453x; 1.0174x over previous
"""Causal self-attention (B=2, T=2048, C=1024, H=16) on 8 TRN2 NeuronCores.

Sharding: data-parallel over batch (2 groups of 4 cores) x tensor-parallel
over heads (4 heads per core). Each core computes qkv for its 4 heads,
causal attention, and a partial output projection (y_heads @ w_proj rows).
Host sums the 4 partial projections per batch and adds b_proj.

Key implementation points (vs the straightforward version):
  - q/k projections run in fp8 e4m3 DoubleRow mode (2 k-subtiles per pass);
    weights are pre-scaled by 64 host-side to stay in e4m3 normal range and
    the 64*64 factor is folded into the exp() scale. v stays bf16.
  - The causal mask is applied on the PE (a -1e9*I @ tri matmul accumulated
    into the score PSUM) instead of a vector op, keeping the PE->ACT->PE
    chain free of DVE hops.
  - v bias is folded into the v PSUM->SBUF copy; attention output bias is
    exact because softmax weights sum to 1.
  - Emission is software-pipelined: qkv for chunk j+1 and projection tiles
    for chunk j-1 are interleaved piecewise into the attention steps of
    chunk j so the PE never starves while the scalar engine runs exp.
  - Partial projections DMA out in bf16 (summed in f32 host-side).
"""
import threading

import numpy as np

import concourse.bass as bass
import concourse.tile as tile
from concourse import bacc, mybir
from concourse.bass2jax import (
    _bass_exec_p,
    install_neuronx_cc_hook,
    partition_id_tensor,
)

N_CORES = 8
B, T, C, H = 2, 2048, 1024, 16
D = C // H            # 64
HL = 4                # heads per core
G = H // HL           # 4 head groups
SC = 1.0 / (32.0 * 4096.0)   # 1/sqrt(C) / (64*64 weight prescale)
F32 = mybir.dt.float32
BF16 = mybir.dt.bfloat16
FP8 = mybir.dt.float8e4
NEG = -1e9
WSCALE = 64.0


def build_nc():
    nc = bacc.Bacc("TRN2", target_bir_lowering=False, debug=False,
                   num_devices=N_CORES)
    x = nc.dram_tensor("x", [128, 4, 8, 512], BF16, kind="ExternalInput").ap()
    x8 = nc.dram_tensor("x8", [128, 4, 8, 512], FP8,
                        kind="ExternalInput").ap()
    wq = nc.dram_tensor("wq", [128, 8, 256], FP8, kind="ExternalInput").ap()
    wk = nc.dram_tensor("wk", [128, 8, 256], FP8, kind="ExternalInput").ap()
    wv = nc.dram_tensor("wv", [128, 8, 256], BF16, kind="ExternalInput").ap()
    bq = nc.dram_tensor("bq", [128, 2], F32, kind="ExternalInput").ap()
    bk = nc.dram_tensor("bk", [128, 2], F32, kind="ExternalInput").ap()
    bv = nc.dram_tensor("bv", [1, 256], F32, kind="ExternalInput").ap()
    wp = nc.dram_tensor("wp", [128, 2, C], BF16, kind="ExternalInput").ap()
    tri = nc.dram_tensor("tri", [128, 2, 128], BF16, kind="ExternalInput").ap()
    neye = nc.dram_tensor("neye", [128, 128], BF16, kind="ExternalInput").ap()
    out = nc.dram_tensor("out", [T, C], BF16, kind="ExternalOutput").ap()

    Exp = mybir.ActivationFunctionType.Exp
    add = mybir.AluOpType.add
    mult = mybir.AluOpType.mult
    DR = mybir.MatmulPerfMode.DoubleRow

    with tile.TileContext(nc) as tc:
        with tc.tile_pool(name="const", bufs=1) as cp, \
             tc.tile_pool(name="ps", bufs=2, space="PSUM") as psp, \
             tc.tile_pool(name="sT", bufs=2, space="PSUM") as stp, \
             tc.tile_pool(name="pvps", bufs=2, space="PSUM") as pvp, \
             tc.tile_pool(name="ptp", bufs=6) as ptp, \
             tc.tile_pool(name="recp", bufs=6) as recp, \
             tc.tile_pool(name="outp", bufs=3) as outp:

            # ---- persistent sbuf (x stored chunk-major: [p, chunk, kt, q])
            xt = cp.tile([128, 4, 8, 512], BF16, tag="xt")   # bf16 (for v)
            x8t = cp.tile([128, 4, 8, 512], FP8, tag="x8t")  # fp8 (for q/k)
            qT = cp.tile([128, 2, T], BF16, tag="qT")        # pair-stacked
            kT = cp.tile([128, 2, T], BF16, tag="kT")
            va = cp.tile([128, 16, HL, D + 1], BF16, tag="va")  # v + ones col
            yt = cp.tile([128, 2, T], BF16, tag="yt")
            wq8 = cp.tile([128, 8, 256], FP8, tag="wq8")
            wk8 = cp.tile([128, 8, 256], FP8, tag="wk8")
            wv16 = cp.tile([128, 8, 256], BF16, tag="wv16")
            wp16 = cp.tile([128, 2, C], BF16, tag="wp16")
            bq_sb = cp.tile([128, 2], F32, tag="bq_sb")
            bk_sb = cp.tile([128, 2], F32, tag="bk_sb")
            bv_row = cp.tile([1, 256], F32, tag="bv_row")
            bv_bc = cp.tile([128, HL, D], F32, tag="bv_bc")
            tri_sb = cp.tile([128, 2, 128], BF16, tag="tri_sb")
            neye_sb = cp.tile([128, 128], BF16, tag="neye_sb")

            # ---- phase 0: loads (inputs pre-cast/pre-swizzled host-side) ----
            # gpsimd queue: weights/consts; sync queue: x streams
            # critical path first: x8 chunk 0 in quarters across three
            # queues, q/k weights + tiny consts on a fourth; bulk xt/x8
            # streams follow behind.
            nc.sync.dma_start(x8t[:, 0, 0:4], x8[:, 0, 0:4])
            nc.scalar.dma_start(x8t[:, 0, 4:8], x8[:, 0, 4:8])
            nc.gpsimd.dma_start(wq8[:], wq[:])
            nc.gpsimd.dma_start(wk8[:], wk[:])
            nc.gpsimd.dma_start(tri_sb[:], tri[:])
            nc.gpsimd.dma_start(neye_sb[:], neye[:])
            nc.gpsimd.dma_start(bq_sb[:], bq[:])
            nc.gpsimd.dma_start(bk_sb[:], bk[:])
            nc.gpsimd.dma_start(bv_row[:], bv[:])
            nc.gpsimd.dma_start(wv16[:], wv[:])
            nc.scalar.dma_start(xt[:, 0], x[:, 0])
            for c in range(1, 4):
                nc.sync.dma_start(x8t[:, c], x8[:, c])
            for c in range(1, 4):
                nc.sync.dma_start(xt[:, c], x[:, c])
            nc.gpsimd.partition_broadcast(
                bv_bc[:].rearrange("p h d -> p (h d)"), bv_row[:])
            nc.gpsimd.dma_start(wp16[:], wp[:])
            nc.vector.memset(va[:, :, :, D:D + 1], 1.0)

            def qk_piece(w8, bias_sb, dst, p, j):
                qs = slice(512 * j, 512 * (j + 1))
                psq = psp.tile([128, 512], F32, tag="ps")
                for dr in range(4):
                    nc.tensor.matmul(
                        psq[:],
                        w8[:, 2 * dr:2 * dr + 2, 128 * p:128 * (p + 1)],
                        x8t[:, j, 2 * dr:2 * dr + 2, :],
                        start=(dr == 0), stop=(dr == 3), perf_mode=DR)
                nc.vector.tensor_scalar_add(
                    dst[:, p, qs], psq[:], bias_sb[:, p:p + 1])

            def v_piece(t):
                psv = psp.tile([128, 512], F32, tag="ps")
                for kt_i in range(8):
                    nc.tensor.matmul(
                        psv[:, :256],
                        xt[:, t // 4, kt_i, 128 * (t % 4):128 * (t % 4 + 1)],
                        wv16[:, kt_i, :],
                        start=(kt_i == 0), stop=(kt_i == 7))
                nc.vector.tensor_tensor(
                    va[:, t, :, 0:D],
                    psv[:, :256].rearrange("p (h d) -> p h d", h=HL),
                    bv_bc[:], add)

            def qkv_pieces(j):
                ps = []
                for (w8, bias_sb, dst) in ((wq8, bq_sb, qT), (wk8, bk_sb, kT)):
                    for p in range(2):
                        ps.append(lambda w8=w8, b=bias_sb, d=dst, p=p:
                                  qk_piece(w8, b, d, p, j))
                for t in range(4 * j, 4 * (j + 1)):
                    ps.append(lambda t=t: v_piece(t))
                return ps

            Copy = mybir.ActivationFunctionType.Copy

            def proj_pieces(j, tail=False):
                def proj_tile(t):
                    osb = outp.tile([128, C], BF16, tag="osb")
                    for cc in range(2):
                        ops = psp.tile([128, 512], F32, tag="ps")
                        for u in range(2):
                            nc.tensor.matmul(
                                ops[:],
                                yt[:, u, 128 * t:128 * (t + 1)],
                                wp16[:, u, 512 * cc:512 * (cc + 1)],
                                start=(u == 0), stop=(u == 1))
                        dst = osb[:, 512 * cc:512 * (cc + 1)]
                        osl = out[128 * t:128 * (t + 1),
                                  512 * cc:512 * (cc + 1)]
                        if tail and cc == 1:
                            nc.scalar.activation(dst, ops[:], Copy)
                            nc.scalar.dma_start(osl, dst)
                        elif tail:
                            nc.vector.tensor_copy(out=dst, in_=ops[:])
                            nc.sync.dma_start(osl, dst)
                        else:
                            nc.any.tensor_copy(out=dst, in_=ops[:])
                    if not tail:
                        nc.sync.dma_start(out[128 * t:128 * (t + 1), :],
                                          osb[:])
                return [lambda t=t: proj_tile(t)
                        for t in range(4 * j, 4 * (j + 1))]

            # ---- chunk 0 q/k up front (v tiles ride the stuffer) ----
            qk_piece(wq8, bq_sb, qT, 0, 0)
            qk_piece(wk8, bk_sb, kT, 0, 0)
            qk_piece(wq8, bq_sb, qT, 1, 0)
            qk_piece(wk8, bk_sb, kT, 1, 0)

            # ---- attention chunks with interleaved qkv/proj stuffing ----
            for j in range(4):
                stuff = []
                if j == 0:
                    stuff += [lambda t=t: v_piece(t) for t in range(4)]
                if j < 3:
                    stuff += qkv_pieces(j + 1)
                if j >= 1:
                    stuff += proj_pieces(j - 1)
                nkb = 4 * j + 4
                nsteps = 2 * nkb
                step = 0
                emitted = 0
                for p in range(2):
                    pvs = [pvp.tile([65, 512], F32, tag="pv", name=f"pv{_h}")
                           for _h in range(2)]
                    pending = []
                    for kb in range(nkb):
                        off = 128 * (kb - 4 * j) if kb >= 4 * j else 0
                        s2 = stp.tile([128, 2, 512], F32, tag="sT")
                        for hh in range(2):
                            pr = 64 * hh
                            nc.tensor.matmul(
                                s2[:, hh, off:512],
                                kT[pr:pr + 64, p, 128 * kb:128 * (kb + 1)],
                                qT[pr:pr + 64, p,
                                   512 * j + off:512 * (j + 1)],
                                start=True, stop=True)
                        if kb >= 4 * j:
                            nc.tensor.matmul(
                                s2[:, :, off:off + 128],
                                neye_sb[:], tri_sb[:],
                                start=False, stop=True,
                                skip_group_check=True)
                        ptt = ptp.tile([128, 2, 512], BF16, tag="pt")
                        nc.scalar.activation(ptt[:, :, off:512],
                                             s2[:, :, off:512], Exp, scale=SC)
                        pending.append((kb, off, ptt))
                        step += 1
                        while emitted * nsteps < len(stuff) * step:
                            stuff[emitted]()
                            emitted += 1
                        if len(pending) > 1:
                            pkb, poff, pptt = pending.pop(0)
                            for hh in range(2):
                                nc.tensor.matmul(
                                    pvs[hh][:, poff:512],
                                    va[:, pkb, 2 * p + hh, :],
                                    pptt[:, hh, poff:512],
                                    start=(pkb == 0), stop=False)
                    for i, (pkb, poff, pptt) in enumerate(pending):
                        for hh in range(2):
                            nc.tensor.matmul(
                                pvs[hh][:, poff:512],
                                va[:, pkb, 2 * p + hh, :],
                                pptt[:, hh, poff:512],
                                start=(pkb == 0),
                                stop=(i == len(pending) - 1))
                    # normalize: yT = y' * (1/rowsum) broadcast over d
                    # (recips first, then broadcasts, then mults, so the
                    # DVE->gpsimd->DVE chains of both heads overlap)
                    recs, bcs = [], []
                    for hh in range(2):
                        rowf = recp.tile([1, 512], F32, tag=f"rowf{hh}")
                        nc.vector.tensor_copy(out=rowf[:],
                                              in_=pvs[hh][64:65, :])
                        rec1 = recp.tile([1, 512], F32, tag=f"rec1{hh}")
                        nc.vector.reciprocal_approx_fast(rec1[:], rowf[:])
                        recs.append(rec1)
                    for hh in range(2):
                        bc = recp.tile([64, 512], F32, tag=f"bc{hh}")
                        nc.gpsimd.partition_broadcast(bc[:], recs[hh][:])
                        bcs.append(bc)
                    for hh in range(2):
                        nc.vector.tensor_tensor(
                            yt[64 * hh:64 * hh + 64, p,
                               512 * j:512 * (j + 1)],
                            pvs[hh][0:64, :], bcs[hh][:], mult)
                while emitted < len(stuff):
                    stuff[emitted]()
                    emitted += 1
            for piece in proj_pieces(3, tail=True):
                piece()

    nc.compile()
    return nc


def make_fn(nc):
    """Sharded 8-core jit callable for the compiled Bass program."""
    import jax
    from jax.sharding import Mesh, PartitionSpec
    from jax.experimental.shard_map import shard_map

    install_neuronx_cc_hook()
    in_names, out_names, out_avals, zero_outs = [], [], [], []
    pname = nc.partition_id_tensor.name if nc.partition_id_tensor else None
    for alloc in nc.m.functions[0].allocations:
        if not isinstance(alloc, mybir.MemoryLocationSet):
            continue
        name = alloc.memorylocations[0].name
        if alloc.kind == "ExternalInput":
            if name != pname:
                in_names.append(name)
        elif alloc.kind == "ExternalOutput":
            out_names.append(name)
            shape = tuple(alloc.tensor_shape)
            dtype = mybir.dt.np(alloc.dtype)
            out_avals.append(jax.core.ShapedArray(shape, dtype))
            zero_outs.append(np.zeros(shape, dtype))
    n_params = len(in_names)
    all_names = list(in_names) + out_names
    if pname is not None:
        all_names.append(pname)

    def _body(*args):
        operands = list(args)
        if pname is not None:
            operands.append(partition_id_tensor())
        outs = _bass_exec_p.bind(
            *operands, out_avals=tuple(out_avals), in_names=tuple(all_names),
            out_names=tuple(out_names), lowering_input_output_aliases=(),
            sim_require_finite=True, sim_require_nnan=True, nc=nc)
        return tuple(outs)

    devices = jax.devices()[:N_CORES]
    mesh = Mesh(np.asarray(devices), ("core",))
    n_out = len(out_names)
    fn = jax.jit(
        shard_map(_body, mesh=mesh,
                  in_specs=(PartitionSpec("core"),) * (n_params + n_out),
                  out_specs=(PartitionSpec("core"),) * n_out,
                  check_rep=False),
        keep_unused=True)
    return fn, in_names, out_names, zero_outs


def shard_inputs(x, w_attn, b_attn, w_proj, b_proj):
    """Build the per-core input maps (core = 4*batch + head_group).

    Host-side prep is layout only: slicing per core, bf16/fp8 rounding, and
    the [128, ko, n] partition swizzle the device matmuls consume."""
    import ml_dtypes
    bf16 = ml_dtypes.bfloat16
    f8 = ml_dtypes.float8_e4m3
    x = np.asarray(x, dtype=np.float32)
    w_attn = np.asarray(w_attn, dtype=np.float32)
    b_attn = np.asarray(b_attn, dtype=np.float32)
    w_proj = np.asarray(w_proj, dtype=np.float32)
    tri = np.where(np.arange(128)[:, None] > np.arange(128)[None, :],
                   np.float32(1.0), np.float32(0.0)).astype(bf16)
    tri2 = np.ascontiguousarray(
        np.broadcast_to(tri[:, None, :], (128, 2, 128)))
    neye = (np.eye(128, dtype=np.float32) * np.float32(NEG)).astype(bf16)
    swz = lambda w, dt: np.ascontiguousarray(
        w.reshape(-1, 128, w.shape[1]).transpose(1, 0, 2).astype(dt))
    # chunk-major x: [p, chunk, kt, q] so per-chunk DMA is contiguous/part
    xsw = [np.ascontiguousarray(
        x[b].T.reshape(8, 128, 4, 512).transpose(1, 2, 0, 3))
        for b in range(B)]
    x16 = [v.astype(bf16) for v in xsw]
    x8 = [v.astype(f8) for v in xsw]
    in_maps = []
    for core in range(N_CORES):
        b, g = divmod(core, G)
        cs = slice(256 * g, 256 * (g + 1))
        bcol = lambda v: np.ascontiguousarray(v.reshape(2, 128).T)
        in_maps.append({
            "x": x16[b],
            "x8": x8[b],
            "wq": swz(w_attn[:, cs] * WSCALE, f8),
            "wk": swz(w_attn[:, 1024:][:, cs] * WSCALE, f8),
            "wv": swz(w_attn[:, 2048:][:, cs], bf16),
            "bq": bcol(b_attn[cs] * WSCALE),
            "bk": bcol(b_attn[1024:][cs] * WSCALE),
            "bv": np.ascontiguousarray(b_attn[2048:][cs]).reshape(1, 256),
            "wp": swz(w_proj[cs, :], bf16),
            "tri": tri2,
            "neye": neye,
        })
    return in_maps


_cache = {}
_lock = threading.Lock()


def _get_compiled():
    with _lock:
        if "fn" not in _cache:
            nc = build_nc()
            fn, in_names, out_names, zero_outs = make_fn(nc)
            _cache.update(fn=fn, nc=nc, in_names=in_names,
                          out_names=out_names, zero_outs=zero_outs)
    return _cache


def run_cores(in_maps):
    """Execute the 8-core SPMD program, return per-core output dicts."""
    import jax

    cc = _get_compiled()
    concat_in = [np.concatenate([m[k] for m in in_maps], axis=0)
                 for k in cc["in_names"]]
    concat_zeros = [np.zeros((N_CORES * z.shape[0], *z.shape[1:]), z.dtype)
                    for z in cc["zero_outs"]]
    outs = cc["fn"](*[jax.device_put(v) for v in concat_in],
                    *[jax.device_put(z) for z in concat_zeros])
    res = []
    for c in range(N_CORES):
        res.append({name: np.asarray(outs[i]).reshape(
            N_CORES, *cc["zero_outs"][i].shape)[c]
            for i, name in enumerate(cc["out_names"])})
    return res


def kernel(x, w_attn, b_attn, w_proj, b_proj):
    in_maps = shard_inputs(x, w_attn, b_attn, w_proj, b_proj)
    res = run_cores(in_maps)
    b_proj = np.asarray(b_proj, dtype=np.float32)
    out = np.empty((B, T, C), dtype=np.float32)
    for b in range(B):
        acc = res[4 * b]["out"].astype(np.float32)
        for g in range(1, G):
            acc = acc + res[4 * b + g]["out"].astype(np.float32)
        out[b] = acc + b_proj
    return out
